# revision 12
# baseline (speedup 1.0000x reference)
"""CSSR classifier kernel for 8 Trainium2 NeuronCores.

Math (per class k):
    h1 = tanh(W1[k] @ xf)          xf: [C=512, B*P=4096]
    h2 = tanh(W2[k] @ h1)
    lt = tanh(W3[k] @ h2)          [L=32, B*P]
    er_raw  = sum_l (lt - proto )^2     -> [B*P]
    er_raw1 = sum_l (lt - proto1)^2
Device returns er_raw per class; host applies  er = max(-0.1*er_raw, -100),
assembles logits and computes the scalar pull/push losses (O(K*B) work).

Sharding: class dim K=100 -> 8 cores x 13 class slots (104, last 4 dummy).
x is replicated; each core holds only its slice of W1/W2/W3/prototypes.
"""

import sys

if "/opt/trn_rl_repo" not in sys.path:
    sys.path.insert(0, "/opt/trn_rl_repo")

import numpy as np
import ml_dtypes

import concourse.bass as bass
import concourse.bacc as bacc
import concourse.mybir as mybir
import concourse.tile as tile
from concourse import bass_utils

BF16 = mybir.dt.bfloat16
F32 = mybir.dt.float32
Tanh = mybir.ActivationFunctionType.Tanh

# problem dims (hardcoded per contract)
B, C, H, W = 64, 512, 8, 8
P = H * W                  # 64 spatial positions
K = 100                    # classes
H1, HID, L = 64, 128, 32
CLIP = 100.0
RED = -0.1
PUSH_THRESH = 10000.0

NCORES = 8
KC = 13                    # class slots per core (8*13 = 104 >= 100)
NPAIR = 7                  # stage-1 class pairs per core (14 slots, #13 dummy)
NB = B * P                 # 4096 free columns
CHUNK = 1024
NCHUNK = NB // CHUNK       # 4

# groups of <=4 classes that share one stage-3/er pack
GROUPS = [
    ([0, 1], [0, 1, 2, 3]),
    ([2, 3], [4, 5, 6, 7]),
    ([4, 5], [8, 9, 10, 11]),
    ([6], [12]),
]

_CACHE = {}


def _build_program():
    nc = bacc.Bacc("TRN2", target_bir_lowering=False, debug=False)

    xf_d = nc.dram_tensor("xf", [C, NB], BF16, kind="ExternalInput").ap()
    w1_d = nc.dram_tensor("w1p", [NPAIR, C, 2 * H1], BF16, kind="ExternalInput").ap()
    # W2[k].T duplicated vertically so lhsT can be based at partition 0 or 64
    # (matmul requires lhsT and rhs to share a base partition)
    w2_d = nc.dram_tensor("w2t", [KC, 2 * H1, HID], BF16, kind="ExternalInput").ap()
    w3_d = nc.dram_tensor("w3t", [KC, HID, L], BF16, kind="ExternalInput").ap()
    # prototypes pre-broadcast over the 16 b's of a chunk: [2, group, 128, CHUNK]
    pr_d = nc.dram_tensor("prb", [2, 4, 128, CHUNK], BF16, kind="ExternalInput").ap()
    ones_d = nc.dram_tensor("onesb", [128, 4], BF16, kind="ExternalInput").ap()
    er0_d = nc.dram_tensor("er0", [KC, NB], F32, kind="ExternalOutput").ap()
    er1_d = nc.dram_tensor("er1", [KC, NB], F32, kind="ExternalOutput").ap()

    with tile.TileContext(nc) as tc:
        with (
            tc.tile_pool(name="weights", bufs=1) as wpool,
            tc.tile_pool(name="xfp", bufs=1) as xfp,
            tc.tile_pool(name="h1p", bufs=4) as h1pool,
            tc.tile_pool(name="h2p", bufs=6) as h2pool,
            tc.tile_pool(name="ltp", bufs=3) as ltpool,
            tc.tile_pool(name="sqp", bufs=4) as sqpool,
            tc.tile_pool(name="ersp", bufs=2) as erspool,
            tc.tile_pool(name="mmp", bufs=3, space="PSUM") as mmpool,
            tc.tile_pool(name="erp", bufs=1, space="PSUM") as erpool,
        ):
            # ---- constants / weights (small, load everything up front) ----
            ones_t = wpool.tile([128, 4], BF16, tag="ones", name="ones_t")
            nc.sync.dma_start(out=ones_t, in_=ones_d)

            w1t = [[None] * 4 for _ in range(NPAIR)]
            for p in range(NPAIR):
                for kk in range(4):
                    t = wpool.tile([128, 2 * H1], BF16, tag=f"w1_{p}_{kk}",
                                   name=f"w1_{p}_{kk}")
                    nc.sync.dma_start(
                        out=t, in_=w1_d[p, 128 * kk:128 * (kk + 1), :])
                    w1t[p][kk] = t
            w2t = []
            for k in range(KC):
                t = wpool.tile([2 * H1, HID], BF16, tag=f"w2_{k}", name=f"w2_{k}")
                nc.sync.dma_start(out=t, in_=w2_d[k])
                w2t.append(t)
            w3t = []
            for k in range(KC):
                t = wpool.tile([HID, L], BF16, tag=f"w3_{k}", name=f"w3_{k}")
                nc.sync.dma_start(out=t, in_=w3_d[k])
                w3t.append(t)
            prt = [[None] * 4 for _ in range(2)]
            for j in range(2):
                for g in range(4):
                    t = wpool.tile([128, CHUNK], BF16, tag=f"pr_{j}_{g}",
                                   name=f"pr_{j}_{g}")
                    nc.sync.dma_start(out=t, in_=pr_d[j, g])
                    prt[j][g] = t

            # ---- x, chunked so compute can start before the full load ----
            xft = [[None] * NCHUNK for _ in range(4)]
            for n in range(NCHUNK):
                for kk in range(4):
                    t = xfp.tile([128, CHUNK], BF16, tag=f"xf_{kk}_{n}",
                                 name=f"xf_{kk}_{n}")
                    nc.sync.dma_start(
                        out=t,
                        in_=xf_d[128 * kk:128 * (kk + 1),
                                 CHUNK * n:CHUNK * (n + 1)])
                    xft[kk][n] = t

            # ---- main loop ----
            for g, (pairs, classes) in enumerate(GROUPS):
                nr = 32 * len(classes)  # active stage-3 rows
                # rows 0:len = er (proto0), rows 32:32+len = er1 (proto1);
                # rows in between are copy slack (free on DVE)
                er_s = erspool.tile([32 + len(classes), NB], F32, tag="ers",
                                    name=f"ers_{g}")
                for n in range(NCHUNK):
                    # stage 1: two classes at a time (M = 2*H1 = 128)
                    h1s = []
                    for pair in pairs:
                        ps1 = mmpool.tile([128, CHUNK], F32, tag="mm",
                                          name=f"ps1_{g}_{n}_{pair}")
                        for kk in range(4):
                            for h in range(2):
                                nc.tensor.matmul(
                                    ps1[:, 512 * h:512 * (h + 1)],
                                    w1t[pair][kk],
                                    xft[kk][n][:, 512 * h:512 * (h + 1)],
                                    start=(kk == 0), stop=(kk == 3))
                        h1 = h1pool.tile([128, CHUNK], BF16, tag="h1",
                                         name=f"h1_{g}_{n}_{pair}")
                        nc.scalar.activation(h1, ps1, Tanh)
                        h1s.append(h1)
                    # stage 2: per class (K = H1 = 64)
                    h2s = []
                    for ci, cls in enumerate(classes):
                        ps2 = mmpool.tile([128, CHUNK], F32, tag="mm",
                                          name=f"ps2_{g}_{n}_{ci}")
                        hsrc = h1s[ci // 2]
                        off = H1 * (ci % 2)
                        for h in range(2):
                            nc.tensor.matmul(
                                ps2[:, 512 * h:512 * (h + 1)],
                                w2t[cls][off:off + H1, :],
                                hsrc[off:off + H1, 512 * h:512 * (h + 1)],
                                start=True, stop=True)
                        h2 = h2pool.tile([128, CHUNK], BF16, tag="h2",
                                         name=f"h2_{g}_{n}_{ci}")
                        nc.scalar.activation(h2, ps2, Tanh)
                        h2s.append(h2)
                    # stage 3: pack up to 4 classes into the 128 partitions
                    ps3 = mmpool.tile([128, CHUNK], F32, tag="mm",
                                      name=f"ps3_{g}_{n}")
                    for ci, cls in enumerate(classes):
                        for h in range(2):
                            nc.tensor.matmul(
                                ps3[32 * ci:32 * (ci + 1), 512 * h:512 * (h + 1)],
                                w3t[cls],
                                h2s[ci][:, 512 * h:512 * (h + 1)],
                                start=True, stop=True,
                                tile_position=(0, 32 * ci))
                    lt = ltpool.tile([128, CHUNK], BF16, tag="lt",
                                     name=f"lt_{g}_{n}")
                    nc.scalar.activation(lt[0:nr, :], ps3[0:nr, :], Tanh)
                    # er path: d = lt - proto ; sq = d*d ; column-sum via PE
                    er_ps = erpool.tile([64, CHUNK], F32, tag="erp",
                                        name=f"erps_{g}_{n}")
                    for j in range(2):
                        d = sqpool.tile([128, CHUNK], BF16, tag="sq",
                                        name=f"d_{g}_{n}_{j}")
                        nc.vector.tensor_sub(d[0:nr, :], lt[0:nr, :],
                                             prt[j][g][0:nr, :])
                        sq = sqpool.tile([128, CHUNK], BF16, tag="sq",
                                         name=f"sq_{g}_{n}_{j}")
                        nc.vector.tensor_mul(sq[0:nr, :], d[0:nr, :], d[0:nr, :])
                        for h in range(2):
                            nc.tensor.matmul(
                                er_ps[32 * j:32 * j + len(classes),
                                      512 * h:512 * (h + 1)],
                                ones_t[0:nr, 0:len(classes)],
                                sq[0:nr, 512 * h:512 * (h + 1)],
                                start=True, stop=True,
                                tile_position=(0, 32 * j))
                    # er = max(raw * RED, -CLIP), fused into the PSUM->SBUF copy
                    for j in range(2):
                        nc.vector.tensor_scalar(
                            er_s[32 * j:32 * j + len(classes),
                                 CHUNK * n:CHUNK * (n + 1)],
                            er_ps[32 * j:32 * j + len(classes), :],
                            RED, -CLIP,
                            op0=mybir.AluOpType.mult, op1=mybir.AluOpType.max)
                nc.sync.dma_start(
                    out=er0_d[4 * g:4 * g + len(classes), :],
                    in_=er_s[0:len(classes), :])
                nc.sync.dma_start(
                    out=er1_d[4 * g:4 * g + len(classes), :],
                    in_=er_s[32:32 + len(classes), :])

    nc.compile()
    return nc


def _prep_in_maps(x, W1, W2, W3, prototypes, prototypes1):
    bf16 = ml_dtypes.bfloat16
    KPAD = NCORES * KC

    x = np.asarray(x, np.float32)
    xf = np.ascontiguousarray(
        x.reshape(B, C, P).transpose(1, 0, 2).reshape(C, NB)).astype(bf16)

    def pad_k(a):
        out = np.zeros((KPAD,) + a.shape[1:], np.float32)
        out[:K] = np.asarray(a, np.float32)
        return out

    W1p = pad_k(W1)                       # [104, H1, C]
    W2p = pad_k(W2)                       # [104, HID, H1]
    W3p = pad_k(W3)                       # [104, L, HID]
    Pr0 = pad_k(np.asarray(prototypes, np.float32).reshape(K, L, P))
    Pr1 = pad_k(np.asarray(prototypes1, np.float32).reshape(K, L, P))

    ones_blk = np.zeros((128, 4), bf16)
    for j in range(4):
        ones_blk[32 * j:32 * (j + 1), j] = 1.0

    in_maps = []
    for c in range(NCORES):
        s = slice(c * KC, (c + 1) * KC)
        w1c = W1p[s].transpose(0, 2, 1)   # [13, C, H1]
        w1c = np.concatenate([w1c, np.zeros((1, C, H1), np.float32)], 0)
        w1pair = np.ascontiguousarray(
            w1c.reshape(NPAIR, 2, C, H1).transpose(0, 2, 1, 3)
            .reshape(NPAIR, C, 2 * H1)).astype(bf16)
        w2c = np.ascontiguousarray(W2p[s].transpose(0, 2, 1)).astype(bf16)
        w2c = np.concatenate([w2c, w2c], axis=1)          # [13, 128, HID]
        w3c = np.ascontiguousarray(W3p[s].transpose(0, 2, 1)).astype(bf16)
        prc = np.zeros((2, 4, 128, CHUNK), np.float32)
        for src, j in ((Pr0[s], 0), (Pr1[s], 1)):
            for g in range(4):
                for jj in range(4):
                    ks = 4 * g + jj
                    if ks < KC:
                        # tile the [L, P] pattern across the 16 b's of a chunk
                        prc[j, g, 32 * jj:32 * (jj + 1)] = np.tile(src[ks], (1, CHUNK // P))
        in_maps.append({
            "xf": xf,
            "w1p": w1pair,
            "w2t": w2c,
            "w3t": w3c,
            "prb": prc.astype(bf16),
            "onesb": ones_blk,
        })
    return in_maps


def _assemble(results, ycls):
    # device already applied  er = max(raw * RED, -CLIP)
    er0 = np.concatenate([r["er0"] for r in results], 0)[:K]   # [100, 4096]
    er1 = np.concatenate([r["er1"] for r in results], 0)[:K]

    logits = np.ascontiguousarray(
        er0.reshape(K, B, P).transpose(1, 0, 2).reshape(B, K, H, W))
    logits1 = np.ascontiguousarray(
        er1.reshape(K, B, P).transpose(1, 0, 2).reshape(B, K, H, W))

    f = er0.reshape(K, B, P).sum(axis=2, dtype=np.float32)     # [K, B]
    f1 = er1.reshape(K, B, P).sum(axis=2, dtype=np.float32)
    ycls = np.asarray(ycls)
    mask_eq = (ycls[None, :] == np.arange(K)[:, None]).astype(np.float32)
    n_eq = mask_eq.sum(axis=1)
    pull = np.where(n_eq > 0, (f1 * mask_eq).sum(axis=1) / np.maximum(n_eq, 1.0),
                    0.0).sum(dtype=np.float32)
    comb = (1.0 - mask_eq) * (f < PUSH_THRESH)
    n_comb = comb.sum(axis=1)
    push = np.where(n_comb > 0, (f * comb).sum(axis=1) / np.maximum(n_comb, 1.0),
                    0.0).sum(dtype=np.float32)
    return logits, logits1, np.float32(pull), np.float32(push)


def kernel_ex(inputs, trace=False):
    """Run the bass kernel; returns ((logits, logits1, pull, push), exec_time_ns)."""
    nc = _CACHE.get("nc")
    if nc is None:
        nc = _build_program()
        _CACHE["nc"] = nc
    in_maps = _prep_in_maps(inputs["x"], inputs["W1"], inputs["W2"],
                            inputs["W3"], inputs["prototypes"],
                            inputs["prototypes1"])
    res = bass_utils.run_bass_kernel_spmd(
        nc, in_maps, core_ids=list(range(NCORES)), trace=trace)
    outs = _assemble(res.results, inputs["ycls"])
    return outs, res.exec_time_ns


def kernel(**inputs):
    outs, _ = kernel_ex(inputs, trace=False)
    return outs


# revision 43
# speedup vs baseline: 21052.3955x; 21052.3955x over previous
"""CSSR classifier kernel for 8 Trainium2 NeuronCores.

Math (per class k):
    h1 = tanh(W1[k] @ xf)          xf: [C=512, B*P=4096]
    h2 = tanh(W2[k] @ h1)
    lt = tanh(W3[k] @ h2)          [L=32, B*P]
    er_raw  = sum_l (lt - proto )^2     -> [B*P]
    er_raw1 = sum_l (lt - proto1)^2
Device returns er_raw per class; host applies  er = max(-0.1*er_raw, -100),
assembles logits and computes the scalar pull/push losses (O(K*B) work).

Sharding: class dim K=100 -> 8 cores x 13 class slots (104, last 4 dummy).
x is replicated; each core holds only its slice of W1/W2/W3/prototypes.
"""

import contextlib
import sys

if "/opt/trn_rl_repo" not in sys.path:
    sys.path.insert(0, "/opt/trn_rl_repo")

import numpy as np
import ml_dtypes

import concourse.bass as bass
import concourse.bacc as bacc
import concourse.mybir as mybir
import concourse.tile as tile
from concourse import bass_utils

BF16 = mybir.dt.bfloat16
F32 = mybir.dt.float32
Tanh = mybir.ActivationFunctionType.Tanh

# problem dims (hardcoded per contract)
B, C, H, W = 64, 512, 8, 8
P = H * W                  # 64 spatial positions
K = 100                    # classes
H1, HID, L = 64, 128, 32
CLIP = 100.0
RED = -0.1
PUSH_THRESH = 10000.0

NCORES = 8
KC = 13                    # class slots per core (8*13 = 104 >= 100)
NPAIR = 7                  # stage-1 class pairs per core (14 slots, #13 dummy)
NB = B * P                 # 4096 free columns
CHUNK = 1024
NCHUNK = NB // CHUNK       # 4

# groups of <=4 classes that share one stage-3/er pack
GROUPS = [
    ([0, 1], [0, 1, 2, 3]),
    ([2, 3], [4, 5, 6, 7]),
    ([4, 5], [8, 9, 10, 11]),
    ([6], [12]),
]

_CACHE = {}
PIPELINE = False


def _build_program(reps=1):
    """reps>1 wraps the compute in a hardware loop (timing builds only)."""
    nc = bacc.Bacc("TRN2", target_bir_lowering=False, debug=False)

    xf_d = nc.dram_tensor("xf", [C, NB], BF16, kind="ExternalInput").ap()
    w1_d = nc.dram_tensor("w1p", [NPAIR, C, 2 * H1], BF16, kind="ExternalInput").ap()
    w2_d = nc.dram_tensor("w2t", [KC, H1, HID], BF16, kind="ExternalInput").ap()
    w3_d = nc.dram_tensor("w3t", [KC, HID, L], BF16, kind="ExternalInput").ap()
    # prototypes per group pack: [2, group, 128, P]; broadcast over the 16
    # b's of a chunk happens in the load DMA via a step-0 AP
    pr_d = nc.dram_tensor("prb", [2, 4, 128, P], BF16, kind="ExternalInput").ap()
    # block "ones" with 16x output replication: ones[l, m] = (l//32 == m//16).
    # The er-reduce matmul then fills all 64 output partitions, so the
    # scale+clip PSUM->SBUF copy is one full-width DVE op per chunk.
    ones_d = nc.dram_tensor("onesb", [128, 64], BF16, kind="ExternalInput").ap()
    er0_d = nc.dram_tensor("er0", [KC, NB], F32, kind="ExternalOutput").ap()
    er1_d = nc.dram_tensor("er1", [KC, NB], F32, kind="ExternalOutput").ap()

    with tile.TileContext(nc) as tc:
        with (
            tc.tile_pool(name="weights", bufs=1) as wpool,
            tc.tile_pool(name="xfp", bufs=1) as xfp,
            tc.tile_pool(name="h1p", bufs=6) as h1pool,
            tc.tile_pool(name="h2p", bufs=8) as h2pool,
            tc.tile_pool(name="ltp", bufs=4) as ltpool,
            tc.tile_pool(name="sqp", bufs=6) as sqpool,
            tc.tile_pool(name="ersp", bufs=3) as erspool,
            tc.tile_pool(name="mmp", bufs=2, space="PSUM") as mmpool,
            tc.tile_pool(name="s3p", bufs=1, space="PSUM") as s3pool,
            tc.tile_pool(name="erp", bufs=1, space="PSUM") as erpool,
        ):
            # ---- DMA issue order matters: the first matmul needs w1[pair0]
            # and the n=0 xf chunk, so those go first; the rest of the
            # weights trail behind, interleaved group-by-group ----
            def load_w1_pair(p):
                tiles = []
                for kk in range(4):
                    t = wpool.tile([128, 2 * H1], BF16, tag=f"w1_{p}_{kk}",
                                   name=f"w1_{p}_{kk}")
                    nc.sync.dma_start(
                        out=t, in_=w1_d[p, 128 * kk:128 * (kk + 1), :])
                    tiles.append(t)
                return tiles

            def load_xf_chunk(n):
                tiles = []
                for kk in range(4):
                    t = xfp.tile([128, CHUNK], BF16, tag=f"xf_{kk}_{n}",
                                 name=f"xf_{kk}_{n}")
                    nc.sync.dma_start(
                        out=t,
                        in_=xf_d[128 * kk:128 * (kk + 1),
                                 CHUNK * n:CHUNK * (n + 1)])
                    tiles.append(t)
                return tiles

            w1t = [None] * NPAIR
            w2t = [None] * KC
            w3t = [None] * KC
            prt = [[None] * 4 for _ in range(2)]
            xft_n = [None] * NCHUNK  # xft_n[n][kk]

            def load_w23(classes):
                for k in classes:
                    # W2[k].T twice (rows 0:64 and 64:128) so lhsT can share
                    # a base partition with either half of the h1 pair tile
                    t = wpool.tile([2 * H1, HID], BF16, tag=f"w2_{k}",
                                   name=f"w2_{k}")
                    nc.sync.dma_start(out=t[0:H1, :], in_=w2_d[k])
                    nc.sync.dma_start(out=t[H1:2 * H1, :], in_=w2_d[k])
                    w2t[k] = t
                    t = wpool.tile([HID, L], BF16, tag=f"w3_{k}", name=f"w3_{k}")
                    nc.sync.dma_start(out=t, in_=w3_d[k])
                    w3t[k] = t

            def load_pr(g):
                for j in range(2):
                    t = wpool.tile([128, CHUNK], BF16, tag=f"pr_{j}_{g}",
                                   name=f"pr_{j}_{g}")
                    src = pr_d[j, g]
                    bcast = bass.AP(
                        tensor=src.tensor, offset=src.offset,
                        ap=[list(src.ap[0]), [0, CHUNK // P], list(src.ap[1])])
                    nc.sync.dma_start(out=t, in_=bcast)
                    prt[j][g] = t

            def load_group_weights(g, pairs, classes):
                for p in pairs:
                    if w1t[p] is None:
                        w1t[p] = load_w1_pair(p)
                load_w23(classes)
                load_pr(g)

            ones_t = wpool.tile([128, 64], BF16, tag="ones", name="ones_t")
            w1t[0] = load_w1_pair(0)
            xft_n[0] = load_xf_chunk(0)
            w1t[1] = load_w1_pair(1)
            load_w23(GROUPS[0][1])
            nc.sync.dma_start(out=ones_t, in_=ones_d)
            xft_n[1] = load_xf_chunk(1)
            load_pr(0)
            load_group_weights(1, GROUPS[1][0], GROUPS[1][1])
            xft_n[2] = load_xf_chunk(2)
            xft_n[3] = load_xf_chunk(3)
            load_group_weights(2, GROUPS[2][0], GROUPS[2][1])
            load_group_weights(3, GROUPS[3][0], GROUPS[3][1])
            xft = [[xft_n[n][kk] for n in range(NCHUNK)] for kk in range(4)]

            # ---- main loop ----
            # Software pipeline: chunk n emits its stage-1/2 work with chunk
            # n-1's stage-3/er work interleaved between the stage-2 matmuls,
            # so PE has independent filler during the ACT-gated PSUM waits.
            def emit_back(g, np_, h2s_prev, er_s):
                """stage-3 + er path for chunk np_ using h2s_prev (list of
                per-class h2 tiles). Returns a generator-like list of
                closures so the caller can interleave them."""
                pairs, classes = GROUPS[g]
                nr = 32 * len(classes)
                mr = 16 * len(classes)
                ps3 = s3pool.tile([128, CHUNK], F32, tag="s3p",
                                  name=f"ps3_{g}_{np_}")

                def s3_for(ci):
                    cls = classes[ci]
                    for h in range(2):
                        nc.tensor.matmul(
                            ps3[32 * ci:32 * (ci + 1), 512 * h:512 * (h + 1)],
                            w3t[cls],
                            h2s_prev[ci][:, 512 * h:512 * (h + 1)],
                            start=True, stop=True,
                            tile_position=(0, 32 * ci))

                def finish():
                    lt = ltpool.tile([128, CHUNK], BF16, tag="lt",
                                     name=f"lt_{g}_{np_}")
                    nc.scalar.activation(lt[0:nr, :], ps3[0:nr, :], Tanh)
                    er_ps = erpool.tile([128, CHUNK], F32, tag="erp",
                                        name=f"erps_{g}_{np_}")
                    for j in range(2):
                        d = sqpool.tile([128, CHUNK], BF16, tag="sq",
                                        name=f"d_{g}_{np_}_{j}")
                        nc.vector.tensor_sub(d[0:nr, :], lt[0:nr, :],
                                             prt[j][g][0:nr, :])
                        sq = sqpool.tile([128, CHUNK], BF16, tag="sq",
                                         name=f"sq_{g}_{np_}_{j}")
                        nc.vector.tensor_mul(sq[0:nr, :], d[0:nr, :],
                                             d[0:nr, :])
                        for h in range(2):
                            nc.tensor.matmul(
                                er_ps[64 * j:64 * j + mr,
                                      512 * h:512 * (h + 1)],
                                ones_t[0:nr, 0:mr],
                                sq[0:nr, 512 * h:512 * (h + 1)],
                                start=True, stop=True,
                                tile_position=(0, 64 * j))
                    # er = max(raw * RED, -CLIP) fused into the PSUM->SBUF copy
                    if nr == 128:
                        nc.vector.tensor_scalar(
                            er_s[:, CHUNK * np_:CHUNK * (np_ + 1)],
                            er_ps[:, :],
                            RED, -CLIP,
                            op0=mybir.AluOpType.mult, op1=mybir.AluOpType.max)
                    else:
                        for j in range(2):
                            nc.vector.tensor_scalar(
                                er_s[64 * j:64 * j + mr,
                                     CHUNK * np_:CHUNK * (np_ + 1)],
                                er_ps[64 * j:64 * j + mr, :],
                                RED, -CLIP,
                                op0=mybir.AluOpType.mult, op1=mybir.AluOpType.max)
                    # per-chunk output DMA keeps the tail short
                    cs = slice(CHUNK * np_, CHUNK * (np_ + 1))
                    nc.sync.dma_start(
                        out=er0_d[4 * g:4 * g + len(classes), cs],
                        in_=er_s[0:16 * len(classes):16, cs])
                    nc.sync.dma_start(
                        out=er1_d[4 * g:4 * g + len(classes), cs],
                        in_=er_s[64:64 + 16 * len(classes):16, cs])

                return s3_for, finish

            def chunk_front(g, n):
                """stage-1 + stage-2 for chunk n; returns h2 tiles."""
                pairs, classes = GROUPS[g]
                h1s = []
                for pair in pairs:
                    ps1 = mmpool.tile([128, CHUNK], F32, tag="mm",
                                      name=f"ps1_{g}_{n}_{pair}")
                    for kk in range(4):
                        for h in range(2):
                            nc.tensor.matmul(
                                ps1[:, 512 * h:512 * (h + 1)],
                                w1t[pair][kk],
                                xft[kk][n][:, 512 * h:512 * (h + 1)],
                                start=(kk == 0), stop=(kk == 3))
                    h1 = h1pool.tile([128, CHUNK], BF16, tag="h1",
                                     name=f"h1_{g}_{n}_{pair}")
                    nc.scalar.activation(h1, ps1, Tanh)
                    h1s.append(h1)
                return h1s

            def s2_class(g, n, ci, h1s):
                pairs, classes = GROUPS[g]
                cls = classes[ci]
                ps2 = mmpool.tile([128, CHUNK], F32, tag="mm",
                                  name=f"ps2_{g}_{n}_{ci}")
                hsrc = h1s[ci // 2]
                off = H1 * (ci % 2)
                for h in range(2):
                    nc.tensor.matmul(
                        ps2[:, 512 * h:512 * (h + 1)],
                        w2t[cls][off:off + H1, :],
                        hsrc[off:off + H1, 512 * h:512 * (h + 1)],
                        start=True, stop=True)
                h2 = h2pool.tile([128, CHUNK], BF16, tag="h2",
                                 name=f"h2_{g}_{n}_{ci}")
                nc.scalar.activation(h2, ps2, Tanh)
                return h2

            # rows 0:64 of er_s = er (proto0, class ci at row 16*ci), rows
            # 64:128 = er1 (proto1); 16x row replication from the
            # ones-matmul, only every 16th row is DMA'd out
            loop_cm = (tc.For_i(0, reps, 1) if reps > 1
                       else contextlib.nullcontext())
            with loop_cm:
              for phase in ([0, 1], [2, 3]):
                ers = {g: erspool.tile([128, NB], F32, tag="ers",
                                       name=f"ers_{g}")
                       for g in phase}
                if PIPELINE:
                    pend = {g: None for g in phase}  # (h2s, n) awaiting back
                    for n in range(NCHUNK + 1):
                        for g in phase:
                            classes = GROUPS[g][1]
                            back = None
                            if pend[g] is not None:
                                h2s_prev, np_ = pend[g]
                                back = emit_back(g, np_, h2s_prev, ers[g])
                            if n < NCHUNK:
                                h1s = chunk_front(g, n)
                                h2s = []
                                for ci in range(len(classes)):
                                    h2s.append(s2_class(g, n, ci, h1s))
                                    if back is not None:
                                        back[0](ci)  # interleave prev s3
                                if back is not None:
                                    back[1]()
                                pend[g] = (h2s, n)
                            else:
                                if back is not None:
                                    for ci in range(len(classes)):
                                        back[0](ci)
                                    back[1]()
                                pend[g] = None
                else:
                    for n in range(NCHUNK):
                        for g in phase:
                            classes = GROUPS[g][1]
                            h1s = chunk_front(g, n)
                            h2s = [s2_class(g, n, ci, h1s)
                                   for ci in range(len(classes))]
                            back = emit_back(g, n, h2s, ers[g])
                            for ci in range(len(classes)):
                                back[0](ci)
                            back[1]()

    nc.compile()
    return nc


def _prep_in_maps(x, W1, W2, W3, prototypes, prototypes1):
    bf16 = ml_dtypes.bfloat16
    KPAD = NCORES * KC

    x = np.asarray(x, np.float32)
    xf = np.ascontiguousarray(
        x.reshape(B, C, P).transpose(1, 0, 2).reshape(C, NB)).astype(bf16)

    def pad_k(a):
        out = np.zeros((KPAD,) + a.shape[1:], np.float32)
        out[:K] = np.asarray(a, np.float32)
        return out

    W1p = pad_k(W1)                       # [104, H1, C]
    W2p = pad_k(W2)                       # [104, HID, H1]
    W3p = pad_k(W3)                       # [104, L, HID]
    Pr0 = pad_k(np.asarray(prototypes, np.float32).reshape(K, L, P))
    Pr1 = pad_k(np.asarray(prototypes1, np.float32).reshape(K, L, P))

    ones_blk = np.zeros((128, 64), bf16)
    for m in range(64):
        ones_blk[32 * (m // 16):32 * (m // 16) + 32, m] = 1.0

    in_maps = []
    for c in range(NCORES):
        s = slice(c * KC, (c + 1) * KC)
        w1c = W1p[s].transpose(0, 2, 1)   # [13, C, H1]
        w1c = np.concatenate([w1c, np.zeros((1, C, H1), np.float32)], 0)
        w1pair = np.ascontiguousarray(
            w1c.reshape(NPAIR, 2, C, H1).transpose(0, 2, 1, 3)
            .reshape(NPAIR, C, 2 * H1)).astype(bf16)
        w2c = np.ascontiguousarray(W2p[s].transpose(0, 2, 1)).astype(bf16)
        w3c = np.ascontiguousarray(W3p[s].transpose(0, 2, 1)).astype(bf16)
        prc = np.zeros((2, 4, 128, P), np.float32)
        for src, j in ((Pr0[s], 0), (Pr1[s], 1)):
            for g in range(4):
                for jj in range(4):
                    ks = 4 * g + jj
                    if ks < KC:
                        prc[j, g, 32 * jj:32 * (jj + 1)] = src[ks]
        in_maps.append({
            "xf": xf,
            "w1p": w1pair,
            "w2t": w2c,
            "w3t": w3c,
            "prb": prc.astype(bf16),
            "onesb": ones_blk,
        })
    return in_maps


def _assemble(results, ycls):
    # device already applied  er = max(raw * RED, -CLIP)
    er0 = np.concatenate([r["er0"] for r in results], 0)[:K]   # [100, 4096]
    er1 = np.concatenate([r["er1"] for r in results], 0)[:K]

    logits = np.ascontiguousarray(
        er0.reshape(K, B, P).transpose(1, 0, 2).reshape(B, K, H, W))
    logits1 = np.ascontiguousarray(
        er1.reshape(K, B, P).transpose(1, 0, 2).reshape(B, K, H, W))

    f = er0.reshape(K, B, P).sum(axis=2, dtype=np.float32)     # [K, B]
    f1 = er1.reshape(K, B, P).sum(axis=2, dtype=np.float32)
    ycls = np.asarray(ycls)
    mask_eq = (ycls[None, :] == np.arange(K)[:, None]).astype(np.float32)
    n_eq = mask_eq.sum(axis=1)
    pull = np.where(n_eq > 0, (f1 * mask_eq).sum(axis=1) / np.maximum(n_eq, 1.0),
                    0.0).sum(dtype=np.float32)
    comb = (1.0 - mask_eq) * (f < PUSH_THRESH)
    n_comb = comb.sum(axis=1)
    push = np.where(n_comb > 0, (f * comb).sum(axis=1) / np.maximum(n_comb, 1.0),
                    0.0).sum(dtype=np.float32)
    return logits, logits1, np.float32(pull), np.float32(push)


def kernel_ex(inputs, trace=False):
    """Run the bass kernel; returns ((logits, logits1, pull, push), exec_time_ns)."""
    nc = _CACHE.get("nc")
    if nc is None:
        nc = _build_program()
        _CACHE["nc"] = nc
    in_maps = _prep_in_maps(inputs["x"], inputs["W1"], inputs["W2"],
                            inputs["W3"], inputs["prototypes"],
                            inputs["prototypes1"])
    res = bass_utils.run_bass_kernel_spmd(
        nc, in_maps, core_ids=list(range(NCORES)), trace=trace)
    outs = _assemble(res.results, inputs["ycls"])
    return outs, res.exec_time_ns


def kernel(**inputs):
    outs, _ = kernel_ex(inputs, trace=False)
    return outs
